# revision 28
# baseline (speedup 1.0000x reference)
"""AUGRU cell (attention-update GRU) Trainium2 Bass kernel.

Problem: h_new = (1-u)*h + u*g with
    u = sigmoid(x@Wxu.T + bxu + h@Whu.T + bhu) * att
    r = sigmoid(x@Wxr.T + bxr + h@Whr.T + bhr)
    g = tanh(x@Wxg.T + bxg + r * (h@Whg.T + bhg))
where inputs = [x | att] with x: [B, 128], att: [B, 1]; h: [B, 128].

Sharding: pure data parallel, batch split across 8 cores (32768 rows each).

Per-core design (one group = 512 batch rows = 4 sub-tiles of 128):
  - "inputs" block loaded fp32 natural [128p, 4t, 129]; h fp32 [128p, 4t, 128].
    Both are PE-transposed (8 transposes) into one 2-bank PSUM tile, then a
    single ACT copy casts both to the matmul dtype: xhT = [xT | hT] bf16.
  - 6 matmuls (weights stationary, N=512) accumulate the 4 gate pre-acts in
    [h, b] layout: psum_u (x+h parts), psum_r, psum_gx, psum_gh.
  - Epilogue in [h, b]:
      u0 = ACT sigmoid(psum_u + bu)          (bias per-partition)
      r  = ACT sigmoid(psum_r + br)
      t1 = DVE stt: (psum_gh + bhg) * r
      t2 = DVE stt: (psum_gx + bxg) + t1
      g  = ACT tanh(t2)
      d  = DVE g - hT
      e  = DVE u0 * d
  - e is PE-transposed back to natural layout (PSUM), and the final DVE stt
    per sub-tile fuses attention + residual: out = (e_nat * att) + h_nat.
  - The emission is software-pipelined (head(i), epilogue(i-1), tail(i-2))
    so no engine's program order stalls on a prior group's late results, and
    PSUM fits: 4 gate banks + 2 transpose banks + 2 e-transpose banks.
"""

import contextlib
import os

import numpy as np

import concourse.bacc as bacc
import concourse.mybir as mybir
from concourse import bass_utils
from concourse.bass import ts
from concourse.masks import make_identity
from concourse.tile import TileContext

B_TOTAL = 262144
N_CORES = 8
BS = B_TOTAL // N_CORES  # rows per core
D = 128
GROUP = 512  # batch rows per group
NT = GROUP // 128  # sub-tiles per group

F32 = mybir.dt.float32
BF16 = mybir.dt.bfloat16

# matmul dtype and epilogue (gate tensors) dtype; bf16 is ~2x faster on the
# bottleneck engines, fp32 is the precision-safe fallback.
MM_DT = BF16 if os.environ.get("AUGRU_MM_DT", "bf16") == "bf16" else F32
EPI_DT = BF16 if os.environ.get("AUGRU_EPI_DT", "bf16") == "bf16" else F32

WKEYS = ["xu", "hu", "xr", "hr", "xg", "hg"]


def augru_tile_kernel(tc, out, inp, h, Ws, Bs, n_rows, repeat=1, loop_repeat=1):
    nc = tc.nc
    n_groups = n_rows // GROUP
    add = mybir.AluOpType.add
    mult = mybir.AluOpType.mult
    Sigmoid = mybir.ActivationFunctionType.Sigmoid
    Tanh = mybir.ActivationFunctionType.Tanh

    with (
        tc.tile_pool(name="consts", bufs=1) as consts,
        tc.tile_pool(name="io", bufs=6) as io,
        tc.tile_pool(name="work", bufs=5) as work,
        tc.tile_pool(name="pgates", bufs=4, space="PSUM") as pgates,
        tc.tile_pool(name="pxh", bufs=1, space="PSUM") as pxh,
        tc.tile_pool(name="pet", bufs=2, space="PSUM") as pet,
    ):
        # ---------- prologue: identities, biases, transposed weights ----------
        ident_f = consts.tile([128, 128], F32, tag="idf", name="ident_f")
        make_identity(nc, ident_f)
        if EPI_DT != F32:
            ident_e = consts.tile([128, 128], EPI_DT, tag="ide", name="ident_e")
            make_identity(nc, ident_e)
        else:
            ident_e = ident_f

        braw = {}
        for k in WKEYS:
            bt = consts.tile([128, 1], F32, tag=f"b{k}", name=f"b{k}_sb")
            nc.sync.dma_start(out=bt, in_=Bs[k])
            braw[k] = bt
        bias_u = consts.tile([128, 1], F32, tag="bias_u", name="bias_u")
        nc.vector.tensor_add(out=bias_u, in0=braw["xu"], in1=braw["hu"])
        bias_r = consts.tile([128, 1], F32, tag="bias_r", name="bias_r")
        nc.vector.tensor_add(out=bias_r, in0=braw["xr"], in1=braw["hr"])
        bias_gx = braw["xg"]
        bias_gh = braw["hg"]

        # all six weights: load (SWDGE, parallel with the HWDGE bias loads),
        # transpose into one psum tile, evacuate with a single copy
        wtmp = {}
        for k in WKEYS:
            wt_in = consts.tile([128, 128], F32, tag=f"wtmp{k}", name=f"wtmp{k}")
            nc.gpsimd.dma_start(out=wt_in, in_=Ws[k])
            wtmp[k] = wt_in
        pw = pxh.tile([128, len(WKEYS), 128], F32, tag="xh", name="pw")
        for i, k in enumerate(WKEYS):
            nc.tensor.transpose(pw[:, i, :], wtmp[k], ident_f)
        WT_all = consts.tile([128, len(WKEYS), 128], MM_DT, tag="WTall", name="WT_all")
        nc.scalar.copy(
            out=WT_all.rearrange("p a b -> p (a b)"),
            in_=pw.rearrange("p a b -> p (a b)"),
        )
        WT = {k: WT_all[:, i, :] for i, k in enumerate(WKEYS)}

        # ---------- software-pipelined group emitters ----------

        def load(b0):
            s = {}
            inb = io.tile([128, NT, D + 1], F32, tag="inb", name="inb")
            nc.sync.dma_start(
                out=inb,
                in_=inp[b0 : b0 + GROUP, :].rearrange("(t p) c -> p t c", p=128),
            )
            hn = io.tile([128, NT, D], F32, tag="hn", name="hn")
            nc.sync.dma_start(
                out=hn,
                in_=h[b0 : b0 + GROUP, :].rearrange("(t p) c -> p t c", p=128),
            )
            s["inb"], s["hn"], s["b0"] = inb, hn, b0
            return s

        def head_a(s):
            inb, hn = s["inb"], s["hn"]

            # [xT | hT] via PE transposes into one 2-bank psum tile, then a
            # single ACT copy(+cast) into SBUF.
            pt = pxh.tile([128, 2, GROUP], F32, tag="xh", name="pt")
            for t in range(NT):
                nc.tensor.transpose(pt[:, 0, ts(t, 128)], inb[:, t, 0:D], ident_f)
            for t in range(NT):
                nc.tensor.transpose(pt[:, 1, ts(t, 128)], hn[:, t, :], ident_f)
            xhT = work.tile([128, 2, GROUP], MM_DT, tag="xhT", name="xhT")
            nc.scalar.copy(
                out=xhT.rearrange("p a b -> p (a b)"),
                in_=pt.rearrange("p a b -> p (a b)"),
            )
            xT = xhT[:, 0, :]
            hT = xhT[:, 1, :]
            s["xT"], s["hT"] = xT, hT

        def head_b(s):
            xT, hT = s["xT"], s["hT"]
            pu = pgates.tile([128, GROUP], F32, tag="gates", name="pu")
            nc.tensor.matmul(pu, WT["xu"], xT, start=True, stop=False)
            nc.tensor.matmul(pu, WT["hu"], hT, start=False, stop=True)
            pr = pgates.tile([128, GROUP], F32, tag="gates", name="pr")
            nc.tensor.matmul(pr, WT["xr"], xT, start=True, stop=False)
            nc.tensor.matmul(pr, WT["hr"], hT, start=False, stop=True)
            pgx = pgates.tile([128, GROUP], F32, tag="gates", name="pgx")
            nc.tensor.matmul(pgx, WT["xg"], xT, start=True, stop=True)
            pgh = pgates.tile([128, GROUP], F32, tag="gates", name="pgh")
            nc.tensor.matmul(pgh, WT["hg"], hT, start=True, stop=True)
            s.update(pu=pu, pr=pr, pgx=pgx, pgh=pgh)

        def epilogue_a(s):
            u0 = work.tile([128, GROUP], EPI_DT, tag="u0", name="u0")
            nc.scalar.activation(out=u0, in_=s["pu"], func=Sigmoid, bias=bias_u)
            r = work.tile([128, GROUP], EPI_DT, tag="r", name="r")
            nc.scalar.activation(out=r, in_=s["pr"], func=Sigmoid, bias=bias_r)

            t1 = work.tile([128, GROUP], F32, tag="t1", name="t1")
            nc.vector.scalar_tensor_tensor(
                out=t1, in0=s["pgh"], scalar=bias_gh, in1=r, op0=add, op1=mult
            )
            t2 = work.tile([128, GROUP], F32, tag="t2", name="t2")
            nc.vector.scalar_tensor_tensor(
                out=t2, in0=s["pgx"], scalar=bias_gx, in1=t1, op0=add, op1=add
            )
            s["u0"], s["t2"] = u0, t2

        def epilogue_b(s):
            gg = work.tile([128, GROUP], EPI_DT, tag="gg", name="gg")
            nc.scalar.activation(out=gg, in_=s["t2"], func=Tanh)
            d = work.tile([128, GROUP], EPI_DT, tag="d", name="d")
            nc.vector.tensor_sub(out=d, in0=gg, in1=s["hT"])
            e = work.tile([128, GROUP], EPI_DT, tag="e", name="e")
            nc.vector.tensor_mul(out=e, in0=s["u0"], in1=d)
            s["e"] = e

        fin_mode = os.environ.get("AUGRU_FIN", "stt")

        def tail(s):
            pe_ = pet.tile([128, GROUP], EPI_DT, tag="et", name="pe_")
            for t in range(NT):
                nc.tensor.transpose(pe_[:, ts(t, 128)], s["e"][:, ts(t, 128)], ident_e)
            inb, hn, b0 = s["inb"], s["hn"], s["b0"]
            f = io.tile([128, NT, D], F32, tag="f", name="f")
            if fin_mode == "ts_pool":
                # f' = att * e_nat on DVE tensor_scalar (2x-capable, bf16
                # psum src), then the +h residual add on the idle GPSIMD.
                fp = io.tile([128, NT, D], EPI_DT, tag="fp", name="fp")
                for t in range(NT):
                    nc.vector.tensor_scalar_mul(
                        fp[:, t, :], pe_[:, ts(t, 128)], inb[:, t, D : D + 1]
                    )
                nc.gpsimd.tensor_add(
                    out=f.rearrange("p t c -> p (t c)"),
                    in0=fp.rearrange("p t c -> p (t c)"),
                    in1=hn.rearrange("p t c -> p (t c)"),
                )
            else:
                for t in range(NT):
                    nc.vector.scalar_tensor_tensor(
                        out=f[:, t, :],
                        in0=pe_[:, ts(t, 128)],
                        scalar=inb[:, t, D : D + 1],
                        in1=hn[:, t, :],
                        op0=mult,
                        op1=add,
                    )
            nc.sync.dma_start(
                out=out[b0 : b0 + GROUP, :].rearrange("(t p) c -> p t c", p=128),
                in_=f,
            )

        # ---------- main loop ----------
        # loop_repeat>1 wraps the body in an on-device For_i; used only by the
        # timing harness (per-execute dispatch overhead through the axon
        # tunnel is ~40-90 ms, so kernel time is measured via the R-slope).
        loop_cm = (
            tc.For_i(0, loop_repeat, 1)
            if loop_repeat > 1
            else contextlib.nullcontext()
        )
        with loop_cm:
            # 6-stage software pipeline; each stage one slot apart so every
            # cross-engine producer finishes a full slot before its consumer:
            #   load(t+1) | headA(t) | headB(t-1) | epiA(t-2) | epiB(t-3) |
            #   tail(t-4)
            n_total = n_groups * repeat
            S = [None] * n_total
            for t in range(n_total + 4):
                if t < n_total:
                    if t == 0:
                        S[0] = load(0)
                    if t + 1 < n_total:
                        S[t + 1] = load(((t + 1) % n_groups) * GROUP)
                    head_a(S[t])
                if 0 <= t - 1 < n_total:
                    head_b(S[t - 1])
                if 0 <= t - 2 < n_total:
                    epilogue_a(S[t - 2])
                if 0 <= t - 3 < n_total:
                    epilogue_b(S[t - 3])
                if 0 <= t - 4 < n_total:
                    tail(S[t - 4])
                    S[t - 4] = None


def build_program(n_rows=BS, repeat=1, loop_repeat=1):
    nc = bacc.Bacc(
        "TRN2", target_bir_lowering=False, debug=False, enable_asserts=False
    )
    inp = nc.dram_tensor("inputs", [n_rows, D + 1], F32, kind="ExternalInput").ap()
    h = nc.dram_tensor("h", [n_rows, D], F32, kind="ExternalInput").ap()
    Ws, Bs = {}, {}
    for k in WKEYS:
        Ws[k] = nc.dram_tensor(f"W{k}", [D, D], F32, kind="ExternalInput").ap()
        Bs[k] = nc.dram_tensor(f"b{k}", [D, 1], F32, kind="ExternalInput").ap()
    out = nc.dram_tensor("out", [n_rows, D], F32, kind="ExternalOutput").ap()

    with TileContext(nc) as tc:
        augru_tile_kernel(
            tc, out, inp, h, Ws, Bs, n_rows, repeat=repeat, loop_repeat=loop_repeat
        )
    nc.compile()
    return nc


_CACHE = {}
LAST_EXEC_NS = None


def kernel(**inputs):
    """Full-input entry point: shards batch across the 8 NeuronCores."""
    global LAST_EXEC_NS
    if "prog" not in _CACHE:
        _CACHE["prog"] = build_program(BS)
    nc = _CACHE["prog"]

    xin = np.ascontiguousarray(np.asarray(inputs["inputs"], dtype=np.float32))
    hin = np.ascontiguousarray(np.asarray(inputs["h"], dtype=np.float32))
    assert xin.shape == (B_TOTAL, D + 1) and hin.shape == (B_TOTAL, D)

    shared = {}
    for k in WKEYS:
        shared[f"W{k}"] = np.ascontiguousarray(
            np.asarray(inputs[f"W{k}"], dtype=np.float32)
        )
        shared[f"b{k}"] = np.ascontiguousarray(
            np.asarray(inputs[f"b{k}"], dtype=np.float32).reshape(D, 1)
        )

    in_maps = []
    for c in range(N_CORES):
        m = dict(shared)
        m["inputs"] = xin[c * BS : (c + 1) * BS]
        m["h"] = hin[c * BS : (c + 1) * BS]
        in_maps.append(m)

    res = bass_utils.run_bass_kernel_spmd(
        nc, in_maps, core_ids=list(range(N_CORES)), trace=False
    )
    LAST_EXEC_NS = res.exec_time_ns
    return np.concatenate([r["out"] for r in res.results], axis=0)
